# revision 3
# baseline (speedup 1.0000x reference)
"""Trainium2 Bass kernel for nn_Decoder_34479997452599.

2-layer LSTM decoder (teacher forcing) + vocab projection + CE loss + greedy preds.

Strategy (8 NeuronCores):
- Gate/hidden tensor-parallel LSTM: core k owns hidden units [k*128,(k+1)*128)
  of both layers (gate slice 512 of 4096). Weights resident in SBUF as f32r
  (fp32 storage, 11-bit-mantissa matmul). Weights are the *moving* matmul
  operand; [x_t; h_prev] is stationary.
- Layer 1 runs one step behind layer 0; one 8-core AllGather per step carries
  [h0_t, h1_{t-1}] slices (f32r).
- Vocab-sharded projection (4000/core) in bf16, block-batched over 128-token
  tiles, fused exp+sum on ScalarE, top-8 argmax candidates on VectorE.
- Host: embedding gather + input sharding; final combine (logsumexp across
  cores, exact target-logit + top-64-candidate argmax refinement in numpy).
"""
import sys
import numpy as np

sys.path.insert(0, "/opt/trn_rl_repo")
import concourse.bass as bass  # noqa: E402
import concourse.mybir as mybir  # noqa: E402
import concourse.tile as tile  # noqa: E402
from concourse import bacc, bass_utils  # noqa: E402
import ml_dtypes  # noqa: E402

NC = 8
B = 64          # batch
E = 512         # embed dim
H = 1024        # hidden dim
V = 32000       # vocab
GS = 512        # gate slice per core (4 gates x 128 hidden)
HS = 128        # hidden slice per core
VS = V // NC    # vocab slice per core (4000)
NSUB = 8        # vocab subtiles per core
SUB = VS // NSUB  # 500

F32 = mybir.dt.float32
F32R = mybir.dt.float32r
BF16 = mybir.dt.bfloat16
U32 = mybir.dt.uint32
AF = mybir.ActivationFunctionType
ALU = mybir.AluOpType
AX = mybir.AxisListType

_BUILD_CACHE = {}


def build(T1):
    """Build the SPMD kernel for T1 LSTM steps."""
    NT = (T1 * B + 127) // 128  # 128-token tiles
    nc = bacc.Bacc("TRN2", target_bir_lowering=False, debug=False,
                   num_devices=NC)

    def din(name, shape, dt):
        return nc.dram_tensor(name, shape, dt, kind="ExternalInput").ap()

    def dout(name, shape, dt):
        return nc.dram_tensor(name, shape, dt, kind="ExternalOutput").ap()

    xT_in = din("xT", [T1, 128, 4 * B], F32R)        # [t, p, c*B+b] = x[t,b,c*128+p]
    wih0_in = din("wih0", [128, 4 * GS], F32R)       # [p, c*GS+j] = W_ih0[gr[j], c*128+p]
    whh0_in = din("whh0", [128, 8 * GS], F32R)
    wih1_in = din("wih1", [128, 8 * GS], F32R)
    whh1_in = din("whh1", [128, 8 * GS], F32R)
    b0_in = din("b0", [1, GS], F32R)
    b1_in = din("b1", [1, GS], F32R)
    wout_in = din("wout", [128, 8 * VS], BF16)       # [p, c*VS+v] = W_out[vs0+v, c*128+p]
    bout_in = din("bout", [1, VS], BF16)
    ones_r_in = din("ones_r", [1, B], F32R)
    ones_b_in = din("ones_b", [1, 128], BF16)
    ident_in = din("ident", [64, 64], F32)

    s_out = dout("s_out", [NT, 128], F32)            # sum(exp(logits)) per token
    v8_out = dout("v8_out", [NT, 128, 8], F32)       # top-8 exp values
    i8_out = dout("i8_out", [NT, 128, 8], U32)       # top-8 local vocab indices
    h1_out = dout("h1_out", [T1, 128, 8, B], F32R)   # h1.T chunks (f32r-rounded)

    with tile.TileContext(nc) as tc:
        with (
            tc.tile_pool(name="wpool", bufs=1) as wpool,
            tc.tile_pool(name="state", bufs=1) as state,
            tc.tile_pool(name="hpool", bufs=2) as hpool,
            tc.tile_pool(name="xpool", bufs=2) as xpool,
            tc.tile_pool(name="actp", bufs=3) as actp,
            tc.tile_pool(name="projp", bufs=2) as projp,
            tc.tile_pool(name="expp", bufs=2) as expp,
            tc.tile_pool(name="statp", bufs=2) as statp,
            tc.tile_pool(name="gps", bufs=2, space="PSUM") as gps,
            tc.tile_pool(name="trps", bufs=2, space="PSUM") as trps,
            tc.tile_pool(name="pps", bufs=2, space="PSUM") as pps,
            tc.tile_pool(name="dram", bufs=2, space="DRAM") as dram,
        ):
            # ---- load resident weights/constants
            wih0 = wpool.tile([128, 4 * GS], F32R, tag="wih0")
            whh0 = wpool.tile([128, 8 * GS], F32R, tag="whh0")
            wih1 = wpool.tile([128, 8 * GS], F32R, tag="wih1")
            whh1 = wpool.tile([128, 8 * GS], F32R, tag="whh1")
            wout = wpool.tile([128, 8 * VS], BF16, tag="wout")
            b0 = wpool.tile([1, GS], F32R, tag="b0")
            b1 = wpool.tile([1, GS], F32R, tag="b1")
            bout = wpool.tile([1, VS], BF16, tag="bout")
            ones_r = wpool.tile([1, B], F32R, tag="ones_r")
            ones_b = wpool.tile([1, 128], BF16, tag="ones_b")
            ident = wpool.tile([64, 64], F32, tag="ident")
            for dst, src in [(wih0, wih0_in), (whh0, whh0_in), (wih1, wih1_in),
                             (whh1, whh1_in), (wout, wout_in), (b0, b0_in),
                             (b1, b1_in), (bout, bout_in), (ones_r, ones_r_in),
                             (ones_b, ones_b_in), (ident, ident_in)]:
                nc.sync.dma_start(dst[:], src)

            # ---- persistent state
            c0 = state.tile([B, HS], F32, tag="c0")
            c1 = state.tile([B, HS], F32, tag="c1")
            zeroT = state.tile([128, B], F32R, tag="zeroT")
            nc.vector.memset(c0[:], 0.0)
            nc.vector.memset(c1[:], 0.0)
            nc.vector.memset(zeroT[:].bitcast(F32), 0.0)

            H0T = None   # h0_{t-1}.T  (128, 8*B) f32r
            H1T = None   # h1_{t-2}.T
            projH = None

            def gate_block(g_ps, c_tile):
                """gates (B,512) psum [i|f|g|o] -> h slice (B,128) sbuf fp32."""
                t_if = actp.tile([B, 256], F32, tag="t_if")
                nc.scalar.activation(t_if[:], g_ps[:, 0:256], AF.Tanh, scale=0.5)
                t_g = actp.tile([B, HS], F32, tag="t_g")
                nc.scalar.activation(t_g[:], g_ps[:, 256:384], AF.Tanh)
                t_o = actp.tile([B, HS], F32, tag="t_o")
                nc.scalar.activation(t_o[:], g_ps[:, 384:512], AF.Tanh, scale=0.5)
                sig_if = actp.tile([B, 256], F32, tag="sig_if")
                nc.vector.tensor_scalar(sig_if[:], t_if[:], 0.5, 0.5, ALU.mult, ALU.add)
                fc = actp.tile([B, HS], F32, tag="fc")
                nc.vector.tensor_mul(out=fc[:], in0=sig_if[:, 128:256], in1=c_tile[:])
                ig = actp.tile([B, HS], F32, tag="ig")
                nc.vector.tensor_mul(out=ig[:], in0=sig_if[:, 0:128], in1=t_g[:])
                nc.vector.tensor_add(out=c_tile[:], in0=fc[:], in1=ig[:])
                t_c = actp.tile([B, HS], F32, tag="t_c")
                nc.scalar.activation(t_c[:], c_tile[:], AF.Tanh)
                sig_o = actp.tile([B, HS], F32, tag="sig_o")
                nc.vector.tensor_scalar(sig_o[:], t_o[:], 0.5, 0.5, ALU.mult, ALU.add)
                h_sl = actp.tile([B, HS], F32, tag="h_sl")
                nc.vector.tensor_mul(out=h_sl[:], in0=sig_o[:], in1=t_c[:])
                return h_sl

            def transpose_slice(h_sl):
                """(B,128) fp32 -> (128,B) f32r."""
                trp = trps.tile([128, B], F32, tag="trp")
                nc.tensor.transpose(trp[:], h_sl[:], ident[:])
                hT = hpool.tile([128, B], F32R, tag="hT")
                nc.vector.tensor_copy(hT[:], trp[:])
                return hT

            def proj_block(tt, pH):
                """Project token tile tt (128 tokens) over this core's vocab slice."""
                exp_sb = expp.tile([128, VS], F32, tag="exp_sb")
                s_parts = statp.tile([128, NSUB], F32, tag="s_parts")
                for sub in range(NSUB):
                    pp = pps.tile([128, 512], F32, tag="pp")
                    vsl = slice(sub * SUB, (sub + 1) * SUB)
                    nc.tensor.matmul(pp[:, 0:SUB], ones_b[:], bout[:, vsl],
                                     start=True, stop=False)
                    for c in range(8):
                        nc.tensor.matmul(
                            pp[:, 0:SUB], pH[:, c * 128:(c + 1) * 128],
                            wout[:, c * VS + sub * SUB: c * VS + (sub + 1) * SUB],
                            start=False, stop=(c == 7))
                    nc.scalar.activation(exp_sb[:, vsl], pp[:, 0:SUB], AF.Exp,
                                         accum_out=s_parts[:, sub:sub + 1])
                s_sb = statp.tile([128, 1], F32, tag="s_sb")
                nc.vector.reduce_sum(s_sb[:], s_parts[:], axis=AX.X)
                v8 = statp.tile([128, 8], F32, tag="v8")
                nc.vector.max(v8[:], exp_sb[:])
                i8 = statp.tile([128, 8], U32, tag="i8")
                nc.vector.max_index(i8[:], v8[:], exp_sb[:])
                nc.sync.dma_start(s_out[tt], s_sb[:])
                nc.sync.dma_start(v8_out[tt], v8[:])
                nc.sync.dma_start(i8_out[tt], i8[:])

            for t in range(T1 + 1):
                do_L0 = t < T1
                do_L1 = t >= 1

                h0T_new = zeroT
                h1T_new = zeroT

                if do_L0:
                    xT = xpool.tile([128, 4 * B], F32R, tag="xT")
                    nc.sync.dma_start(xT[:], xT_in[t])
                    g0 = gps.tile([B, GS], F32, tag="g0")
                    nc.tensor.matmul(g0[:], ones_r[:], b0[:], start=True, stop=False)
                    for c in range(4):
                        nc.tensor.matmul(g0[:], xT[:, c * B:(c + 1) * B],
                                         wih0[:, c * GS:(c + 1) * GS],
                                         start=False, stop=(t == 0 and c == 3))
                    if t > 0:
                        for c in range(8):
                            nc.tensor.matmul(g0[:], H0T[:, c * B:(c + 1) * B],
                                             whh0[:, c * GS:(c + 1) * GS],
                                             start=False, stop=(c == 7))
                    h0_sl = gate_block(g0, c0)
                    h0T_new = transpose_slice(h0_sl)

                if do_L1:
                    g1 = gps.tile([B, GS], F32, tag="g1")
                    nc.tensor.matmul(g1[:], ones_r[:], b1[:], start=True, stop=False)
                    for c in range(8):
                        nc.tensor.matmul(g1[:], H0T[:, c * B:(c + 1) * B],
                                         wih1[:, c * GS:(c + 1) * GS],
                                         start=False, stop=(t == 1 and c == 7))
                    if t > 1:
                        for c in range(8):
                            nc.tensor.matmul(g1[:], H1T[:, c * B:(c + 1) * B],
                                             whh1[:, c * GS:(c + 1) * GS],
                                             start=False, stop=(c == 7))
                    h1_sl = gate_block(g1, c1)
                    h1T_new = transpose_slice(h1_sl)

                # ---- AllGather [h0_t, h1_{t-1}] slices
                ag_in = dram.tile([2, 128, B], F32R, tag="ag_in")
                nc.sync.dma_start(ag_in[0], h0T_new[:])
                nc.sync.dma_start(ag_in[1], h1T_new[:])
                ag_out = dram.tile([2 * NC, 128, B], F32R, tag="ag_out",
                                   addr_space="Shared")
                nc.gpsimd.collective_compute(
                    "AllGather", ALU.bypass,
                    replica_groups=[list(range(NC))],
                    ins=[ag_in.opt()], outs=[ag_out.opt()])
                agv = ag_out[:].rearrange("(c two) p b -> two p c b", two=2)

                if do_L0:
                    H0T_t = hpool.tile([128, 8 * B], F32R, tag="H0T")
                    nc.sync.dma_start(
                        H0T_t[:].rearrange("p (c b) -> p c b", c=8), agv[0])
                    H0T = H0T_t
                if do_L1:
                    H1T_t = hpool.tile([128, 8 * B], F32R, tag="H1T")
                    nc.sync.dma_start(
                        H1T_t[:].rearrange("p (c b) -> p c b", c=8), agv[1])
                    H1T = H1T_t
                    # dump h1_{t-1} for the host
                    nc.sync.dma_start(h1_out[t - 1], agv[1])
                    # feed the projection buffer (bf16 cast)
                    s = t - 1
                    ph, tt = s % 2, s // 2
                    if ph == 0:
                        projH = projp.tile([128, 8 * 128], BF16, tag="projH")
                    nc.vector.tensor_copy(
                        projH[:].rearrange("p (c k) -> p c k", c=8)
                        [:, :, ph * B:(ph + 1) * B],
                        H1T_t[:].rearrange("p (c b) -> p c b", c=8))
                    if ph == 1 or s == T1 - 1:
                        proj_block(tt, projH)

    nc.compile()
    return nc


def _round11(x):
    return x  # device rounds f32r internally; host passes fp32 bits


def _prep_inputs(output_tensor, emb, W_ih0, W_hh0, b_ih0, b_hh0,
                 W_ih1, W_hh1, b_ih1, b_hh1, W_out, b_out, T1):
    """Build per-core in_maps (host-side sharding)."""
    tok = np.asarray(output_tensor)[:T1]
    x = emb[tok] * (tok != 0)[..., None].astype(np.float32)      # (T1,B,E)
    xT = np.ascontiguousarray(
        x.reshape(T1, B, 4, 128).transpose(0, 3, 2, 1)).reshape(T1, 128, 4 * B)

    def pack_w(Wsl, nch):
        # Wsl (512 gate rows, K) -> [p, c*GS+j] = Wsl[j, c*128+p]
        K = Wsl.shape[1]
        arr = Wsl.T.reshape(nch, 128, GS).transpose(1, 0, 2).reshape(128, nch * GS)
        assert K == nch * 128
        return np.ascontiguousarray(arr)

    ones_r = np.ones((1, B), np.float32)
    ones_b = np.ones((1, 128), ml_dtypes.bfloat16)
    ident = np.eye(64, dtype=np.float32)

    in_maps = []
    for k in range(NC):
        gr = np.concatenate([np.arange(q * H + k * HS, q * H + (k + 1) * HS)
                             for q in range(4)])
        vsl = slice(k * VS, (k + 1) * VS)
        woutT = (W_out[vsl].T.reshape(8, 128, VS).transpose(1, 0, 2)
                 .reshape(128, 8 * VS))
        in_maps.append({
            "xT": xT,
            "wih0": pack_w(W_ih0[gr], 4),
            "whh0": pack_w(W_hh0[gr], 8),
            "wih1": pack_w(W_ih1[gr], 8),
            "whh1": pack_w(W_hh1[gr], 8),
            "b0": (b_ih0 + b_hh0)[gr][None, :].astype(np.float32),
            "b1": (b_ih1 + b_hh1)[gr][None, :].astype(np.float32),
            "wout": np.ascontiguousarray(woutT).astype(ml_dtypes.bfloat16),
            "bout": b_out[vsl][None, :].astype(ml_dtypes.bfloat16),
            "ones_r": ones_r,
            "ones_b": ones_b,
            "ident": ident,
        })
    return in_maps


def _combine(results, output_tensor, W_out, b_out, T, T1):
    """Host-side unshard: loss + greedy preds from per-core stats."""
    Ntok = T1 * B
    NT = (Ntok + 127) // 128
    # global sumexp -> logsumexp
    s_g = np.zeros((NT * 128,), np.float64)
    for r in results:
        s_g += r["s_out"].reshape(-1).astype(np.float64)
    lse = np.log(s_g[:Ntok]).reshape(T1, B)

    # h1 as the device used it (f32r-rounded): (T1,128,8,B)->(T1,B,H)
    h1 = results[0]["h1_out"].transpose(0, 3, 2, 1).reshape(T1, B, H)
    h1 = np.ascontiguousarray(h1, np.float32)

    tgt = np.asarray(output_tensor)[1:T1 + 1]
    tv = (np.einsum("tbh,tbh->tb", h1.astype(np.float64),
                    W_out[tgt].astype(np.float64))
          + b_out[tgt].astype(np.float64))
    nll = lse - tv
    valid = (tgt != 1)
    denom = np.maximum(valid.sum(axis=1), 1.0)
    ce = (nll * valid).sum(axis=1) / denom
    loss = np.float32(ce.sum() / T)

    # candidate-refined argmax
    cand = np.zeros((NT * 128, NC * 8), np.int64)
    for k, r in enumerate(results):
        cand[:, k * 8:(k + 1) * 8] = (
            r["i8_out"].reshape(-1, 8).astype(np.int64) + k * VS)
    cand = np.sort(cand[:Ntok], axis=1)
    h1f = h1.reshape(Ntok, H)
    preds = np.zeros((Ntok,), np.int64)
    CHUNK = 512
    for i in range(0, Ntok, CHUNK):
        c = cand[i:i + CHUNK]
        wl = W_out[c]                                   # (n, 64, H)
        cl = np.einsum("nkh,nh->nk", wl, h1f[i:i + CHUNK]) + b_out[c]
        preds[i:i + CHUNK] = c[np.arange(len(c)), np.argmax(cl, axis=1)]
    preds = preds.reshape(T1, B).astype(np.int32)
    result = np.concatenate(
        [np.full((1, B), 2, np.int32), preds], axis=0)
    return loss, result


def kernel(output_tensor, encoder_hidden_states=None, input_mask=None,
           hidden_state=None, cell_state=None, encoder_attention=None,
           max_length=None, emb=None, W_ih0=None, W_hh0=None, b_ih0=None,
           b_hh0=None, W_ih1=None, W_hh1=None, b_ih1=None, b_hh1=None,
           W_out=None, b_out=None):
    output_tensor = np.asarray(output_tensor)
    T = output_tensor.shape[0]
    T1 = T - 1
    emb = np.asarray(emb, np.float32)
    W_ih0 = np.asarray(W_ih0, np.float32); W_hh0 = np.asarray(W_hh0, np.float32)
    W_ih1 = np.asarray(W_ih1, np.float32); W_hh1 = np.asarray(W_hh1, np.float32)
    b_ih0 = np.asarray(b_ih0, np.float32); b_hh0 = np.asarray(b_hh0, np.float32)
    b_ih1 = np.asarray(b_ih1, np.float32); b_hh1 = np.asarray(b_hh1, np.float32)
    W_out = np.asarray(W_out, np.float32); b_out = np.asarray(b_out, np.float32)

    if T1 not in _BUILD_CACHE:
        _BUILD_CACHE[T1] = build(T1)
    nc = _BUILD_CACHE[T1]

    in_maps = _prep_inputs(output_tensor, emb, W_ih0, W_hh0, b_ih0, b_hh0,
                           W_ih1, W_hh1, b_ih1, b_hh1, W_out, b_out, T1)
    res = bass_utils.run_bass_kernel_spmd(nc, in_maps, core_ids=list(range(NC)))
    return _combine(res.results, output_tensor, W_out, b_out, T, T1)


# revision 12
# speedup vs baseline: 2163.9435x; 2163.9435x over previous
"""Trainium2 Bass kernel for nn_Decoder_34479997452599.

2-layer LSTM decoder (teacher forcing) + vocab projection + CE loss + greedy preds.

Strategy (8 NeuronCores):
- Gate/hidden tensor-parallel LSTM: core k owns hidden units [k*128,(k+1)*128)
  of both layers (gate slice 512 of 4096). Weights resident in SBUF as f32r
  (fp32 storage, 11-bit-mantissa matmul). Weights are the *moving* matmul
  operand; [x_t; h_prev] is stationary.
- Layer 1 runs one step behind layer 0; one 8-core AllGather per step carries
  [h0_t, h1_{t-1}] slices (f32r).
- Vocab-sharded projection (4000/core) in bf16, block-batched over 128-token
  tiles, fused exp+sum on ScalarE, top-8 argmax candidates on VectorE.
- Host: embedding gather + input sharding; final combine (logsumexp across
  cores, exact target-logit + top-64-candidate argmax refinement in numpy).
"""
import os
import sys
import numpy as np

sys.path.insert(0, "/opt/trn_rl_repo")
import concourse.bass as bass  # noqa: E402
import concourse.mybir as mybir  # noqa: E402
import concourse.tile as tile  # noqa: E402
from concourse import bacc, bass_utils  # noqa: E402
import ml_dtypes  # noqa: E402

NC = 8
B = 64          # batch
E = 512         # embed dim
H = 1024        # hidden dim
V = 32000       # vocab
GS = 512        # gate slice per core (4 gates x 128 hidden)
HS = 128        # hidden slice per core
VS = V // NC    # vocab slice per core (4000)
NSUB = 8        # vocab subtiles per core
SUB = VS // NSUB  # 500

F32 = mybir.dt.float32
F32R = mybir.dt.float32r
BF16 = mybir.dt.bfloat16
U32 = mybir.dt.uint32
AF = mybir.ActivationFunctionType
ALU = mybir.AluOpType
AX = mybir.AxisListType

_BUILD_CACHE = {}


def build(T1):
    """Build the SPMD kernel for T1 LSTM steps."""
    NT = (T1 * B + 127) // 128  # 128-token tiles
    nc = bacc.Bacc("TRN2", target_bir_lowering=False, debug=False,
                   num_devices=NC)

    def din(name, shape, dt):
        return nc.dram_tensor(name, shape, dt, kind="ExternalInput").ap()

    def dout(name, shape, dt):
        return nc.dram_tensor(name, shape, dt, kind="ExternalOutput").ap()

    xT_in = din("xT", [T1, 128, 4 * B], F32R)        # [t, p, c*B+b] = x[t,b,c*128+p]
    wih0_in = din("wih0", [128, 4 * GS], F32R)       # [p, c*GS+j] = W_ih0[gr[j], c*128+p]
    whh0_in = din("whh0", [128, 8 * GS], F32R)
    wih1_in = din("wih1", [128, 8 * GS], F32R)
    whh1_in = din("whh1", [128, 8 * GS], F32R)
    b0_in = din("b0", [1, GS], F32R)
    b1_in = din("b1", [1, GS], F32R)
    wout_in = din("wout", [128, 8 * VS], BF16)       # [p, c*VS+v] = W_out[vs0+v, c*128+p]
    bout_in = din("bout", [1, VS], BF16)
    ones_r_in = din("ones_r", [1, B], F32R)
    ones_b_in = din("ones_b", [1, 128], BF16)
    ident_in = din("ident", [64, 64], F32)

    s_out = dout("s_out", [NT, 128], F32)            # sum(exp(logits)) per token
    v8_out = dout("v8_out", [NT, 128, 8], F32)       # top-8 exp values
    i8_out = dout("i8_out", [NT, 128, 8], U32)       # top-8 local vocab indices
    h1_out = dout("h1_out", [T1, 128, 8, B], F32R)   # h1.T chunks (f32r-rounded)

    with tile.TileContext(nc) as tc:
        with (
            tc.tile_pool(name="wpool", bufs=1) as wpool,
            tc.tile_pool(name="state", bufs=1) as state,
            tc.tile_pool(name="hpool", bufs=2) as hpool,
            tc.tile_pool(name="xpool", bufs=2) as xpool,
            tc.tile_pool(name="actp", bufs=3) as actp,
            tc.tile_pool(name="projp", bufs=2) as projp,
            tc.tile_pool(name="expp", bufs=2) as expp,
            tc.tile_pool(name="statp", bufs=2) as statp,
            tc.tile_pool(name="gps", bufs=2, space="PSUM") as gps,
            tc.tile_pool(name="trps", bufs=2, space="PSUM") as trps,
            tc.tile_pool(name="pps", bufs=2, space="PSUM") as pps,
            tc.tile_pool(name="dram", bufs=2, space="DRAM") as dram,
        ):
            # ---- load resident weights/constants
            wih0 = wpool.tile([128, 4 * GS], F32R, tag="wih0")
            whh0 = wpool.tile([128, 8 * GS], F32R, tag="whh0")
            wih1 = wpool.tile([128, 8 * GS], F32R, tag="wih1")
            whh1 = wpool.tile([128, 8 * GS], F32R, tag="whh1")
            wout = wpool.tile([128, 8 * VS], BF16, tag="wout")
            b0 = wpool.tile([1, GS], F32R, tag="b0")
            b1 = wpool.tile([1, GS], F32R, tag="b1")
            bout = wpool.tile([1, VS], BF16, tag="bout")
            ones_r = wpool.tile([1, B], F32R, tag="ones_r")
            ones_b = wpool.tile([1, 128], BF16, tag="ones_b")
            ident = wpool.tile([64, 64], F32, tag="ident")
            for dst, src in [(wih0, wih0_in), (whh0, whh0_in), (wih1, wih1_in),
                             (whh1, whh1_in), (wout, wout_in), (b0, b0_in),
                             (b1, b1_in), (bout, bout_in), (ones_r, ones_r_in),
                             (ones_b, ones_b_in), (ident, ident_in)]:
                nc.sync.dma_start(dst[:], src)

            # ---- persistent state
            c0 = state.tile([B, HS], F32, tag="c0")
            c1 = state.tile([B, HS], F32, tag="c1")
            zeroT = state.tile([128, B], F32R, tag="zeroT")
            nc.vector.memset(c0[:], 0.0)
            nc.vector.memset(c1[:], 0.0)
            nc.vector.memset(zeroT[:].bitcast(F32), 0.0)

            H0T = None   # h0_{t-1}.T  (128, 8*B) f32r
            H1T = None   # h1_{t-2}.T
            projH = None

            def gate_block(g_ps, c_tile):
                """gates (B,512) psum [i|f|g|o] -> h slice (B,128) sbuf fp32."""
                t_if = actp.tile([B, 256], F32, tag="t_if")
                nc.scalar.activation(t_if[:], g_ps[:, 0:256], AF.Tanh, scale=0.5)
                t_g = actp.tile([B, HS], F32, tag="t_g")
                nc.scalar.activation(t_g[:], g_ps[:, 256:384], AF.Tanh)
                t_o = actp.tile([B, HS], F32, tag="t_o")
                nc.scalar.activation(t_o[:], g_ps[:, 384:512], AF.Tanh, scale=0.5)
                sig_if = actp.tile([B, 256], F32, tag="sig_if")
                nc.vector.tensor_scalar(sig_if[:], t_if[:], 0.5, 0.5, ALU.mult, ALU.add)
                fc = actp.tile([B, HS], F32, tag="fc")
                nc.vector.tensor_mul(out=fc[:], in0=sig_if[:, 128:256], in1=c_tile[:])
                ig = actp.tile([B, HS], F32, tag="ig")
                nc.vector.tensor_mul(out=ig[:], in0=sig_if[:, 0:128], in1=t_g[:])
                nc.vector.tensor_add(out=c_tile[:], in0=fc[:], in1=ig[:])
                t_c = actp.tile([B, HS], F32, tag="t_c")
                nc.scalar.activation(t_c[:], c_tile[:], AF.Tanh)
                sig_o = actp.tile([B, HS], F32, tag="sig_o")
                nc.vector.tensor_scalar(sig_o[:], t_o[:], 0.5, 0.5, ALU.mult, ALU.add)
                h_sl = actp.tile([B, HS], F32, tag="h_sl")
                nc.vector.tensor_mul(out=h_sl[:], in0=sig_o[:], in1=t_c[:])
                return h_sl

            def transpose_slice(h_sl):
                """(B,128) fp32 -> (128,B) f32r."""
                trp = trps.tile([128, B], F32, tag="trp")
                nc.tensor.transpose(trp[:], h_sl[:], ident[:])
                hT = hpool.tile([128, B], F32R, tag="hT")
                nc.vector.tensor_copy(hT[:], trp[:])
                return hT

            def proj_block(tt, pH):
                """Project token tile tt (128 tokens) over this core's vocab slice."""
                exp_sb = expp.tile([128, VS], F32, tag="exp_sb")
                s_parts = statp.tile([128, NSUB], F32, tag="s_parts")
                for sub in range(NSUB):
                    pp = pps.tile([128, 512], F32, tag="pp")
                    vsl = slice(sub * SUB, (sub + 1) * SUB)
                    nc.tensor.matmul(pp[:, 0:SUB], ones_b[:], bout[:, vsl],
                                     start=True, stop=False)
                    for c in range(8):
                        nc.tensor.matmul(
                            pp[:, 0:SUB], pH[:, c * 128:(c + 1) * 128],
                            wout[:, c * VS + sub * SUB: c * VS + (sub + 1) * SUB],
                            start=False, stop=(c == 7))
                    nc.scalar.activation(exp_sb[:, vsl], pp[:, 0:SUB], AF.Exp,
                                         accum_out=s_parts[:, sub:sub + 1])
                s_sb = statp.tile([128, 1], F32, tag="s_sb")
                nc.vector.reduce_sum(s_sb[:], s_parts[:], axis=AX.X)
                v8 = statp.tile([128, 8], F32, tag="v8")
                nc.vector.max(v8[:], exp_sb[:])
                i8 = statp.tile([128, 8], U32, tag="i8")
                nc.vector.max_index(i8[:], v8[:], exp_sb[:])
                nc.sync.dma_start(s_out[tt], s_sb[:])
                nc.sync.dma_start(v8_out[tt], v8[:])
                nc.sync.dma_start(i8_out[tt], i8[:])

            for t in range(T1 + 1):
                do_L0 = t < T1
                do_L1 = t >= 1

                h0T_new = zeroT
                h1T_new = zeroT

                if do_L0:
                    xT = xpool.tile([128, 4 * B], F32R, tag="xT")
                    nc.sync.dma_start(xT[:], xT_in[t])
                    g0 = gps.tile([B, GS], F32, tag="g0")
                    nc.tensor.matmul(g0[:], ones_r[:], b0[:], start=True, stop=False)
                    for c in range(4):
                        nc.tensor.matmul(g0[:], xT[:, c * B:(c + 1) * B],
                                         wih0[:, c * GS:(c + 1) * GS],
                                         start=False, stop=(t == 0 and c == 3))
                    if t > 0:
                        for c in range(8):
                            nc.tensor.matmul(g0[:], H0T[:, c * B:(c + 1) * B],
                                             whh0[:, c * GS:(c + 1) * GS],
                                             start=False, stop=(c == 7))
                    h0_sl = gate_block(g0, c0)
                    h0T_new = transpose_slice(h0_sl)

                if do_L1:
                    g1 = gps.tile([B, GS], F32, tag="g1")
                    nc.tensor.matmul(g1[:], ones_r[:], b1[:], start=True, stop=False)
                    for c in range(8):
                        nc.tensor.matmul(g1[:], H0T[:, c * B:(c + 1) * B],
                                         wih1[:, c * GS:(c + 1) * GS],
                                         start=False, stop=(t == 1 and c == 7))
                    if t > 1:
                        for c in range(8):
                            nc.tensor.matmul(g1[:], H1T[:, c * B:(c + 1) * B],
                                             whh1[:, c * GS:(c + 1) * GS],
                                             start=False, stop=(c == 7))
                    h1_sl = gate_block(g1, c1)
                    h1T_new = transpose_slice(h1_sl)

                # ---- AllGather [h0_t, h1_{t-1}] slices
                ag_in = dram.tile([2, 128, B], F32R, tag="ag_in")
                nc.sync.dma_start(ag_in[0], h0T_new[:])
                nc.sync.dma_start(ag_in[1], h1T_new[:])
                ag_out = dram.tile([2 * NC, 128, B], F32R, tag="ag_out",
                                   addr_space="Shared")
                nc.gpsimd.collective_compute(
                    "AllGather", ALU.bypass,
                    replica_groups=[list(range(NC))],
                    ins=[ag_in.opt()], outs=[ag_out.opt()])
                agv = ag_out[:].rearrange("(c two) p b -> two p c b", two=2)

                if do_L0:
                    H0T_t = hpool.tile([128, 8 * B], F32R, tag="H0T")
                    nc.sync.dma_start(
                        H0T_t[:].rearrange("p (c b) -> p c b", c=8), agv[0])
                    H0T = H0T_t
                if do_L1:
                    H1T_t = hpool.tile([128, 8 * B], F32R, tag="H1T")
                    nc.sync.dma_start(
                        H1T_t[:].rearrange("p (c b) -> p c b", c=8), agv[1])
                    H1T = H1T_t
                    # dump h1_{t-1} for the host
                    nc.sync.dma_start(h1_out[t - 1], agv[1])
                    # feed the projection buffer (bf16 cast)
                    s = t - 1
                    ph, tt = s % 2, s // 2
                    if ph == 0:
                        projH = projp.tile([128, 8 * 128], BF16, tag="projH")
                    nc.vector.tensor_copy(
                        projH[:].rearrange("p (c k) -> p c k", c=8)
                        [:, :, ph * B:(ph + 1) * B],
                        H1T_t[:].rearrange("p (c b) -> p c b", c=8))
                    if ph == 1 or s == T1 - 1:
                        proj_block(tt, projH)

    nc.compile()
    return nc


def make_runner(nc):
    """Persistent PJRT runner for the SPMD kernel.

    Returns (put_inputs, run, out_names):
      put_inputs(in_maps) -> device-resident global input arrays
      run(dev_ins) -> tuple of global output jax Arrays (blocked until ready)
    Donated output buffers are zero-filled on device each call, so repeated
    runs move no host<->device data beyond the first upload.
    """
    import jax
    import jax.numpy as jnp
    from jax.experimental.shard_map import shard_map
    from jax.sharding import Mesh, NamedSharding, PartitionSpec

    from concourse import bass2jax

    bass2jax.install_neuronx_cc_hook()
    partition_name = (nc.partition_id_tensor.name
                      if nc.partition_id_tensor else None)
    in_names, out_names, out_avals, zero_shapes = [], [], [], []
    for alloc in nc.m.functions[0].allocations:
        if not isinstance(alloc, mybir.MemoryLocationSet):
            continue
        name = alloc.memorylocations[0].name
        if alloc.kind == "ExternalInput":
            if name != partition_name:
                in_names.append(name)
        elif alloc.kind == "ExternalOutput":
            shape = tuple(alloc.tensor_shape)
            dtype = mybir.dt.np(alloc.dtype)
            out_names.append(name)
            out_avals.append(jax.core.ShapedArray(shape, dtype))
            zero_shapes.append((shape, dtype))
    n_params = len(in_names)
    all_in_names = list(in_names) + list(out_names) + (
        [partition_name] if partition_name else [])
    donate = tuple(range(n_params, n_params + len(out_names)))

    def _body(*args):
        operands = list(args)
        if partition_name is not None:
            operands.append(bass2jax.partition_id_tensor())
        outs = bass2jax._bass_exec_p.bind(
            *operands, out_avals=tuple(out_avals),
            in_names=tuple(all_in_names), out_names=tuple(out_names),
            lowering_input_output_aliases=(),
            sim_require_finite=True, sim_require_nnan=True, nc=nc)
        return tuple(outs)

    devices = jax.devices()[:NC]
    mesh = Mesh(np.asarray(devices), ("core",))
    in_specs = (PartitionSpec("core"),) * (n_params + len(out_names))
    out_specs = (PartitionSpec("core"),) * len(out_names)
    sharded = jax.jit(
        shard_map(_body, mesh=mesh, in_specs=in_specs, out_specs=out_specs,
                  check_rep=False),
        donate_argnums=donate, keep_unused=True)
    sh = NamedSharding(mesh, PartitionSpec("core"))
    zmaker = jax.jit(
        lambda: tuple(jnp.zeros((NC * s[0],) + tuple(s[1:]), d)
                      for s, d in zero_shapes),
        out_shardings=(sh,) * len(zero_shapes))

    def put_inputs(in_maps):
        return [jax.device_put(
            np.concatenate([np.asarray(m[n]) for m in in_maps], axis=0), sh)
            for n in in_names]

    def run(dev_ins):
        outs = sharded(*dev_ins, *zmaker())
        jax.block_until_ready(outs)
        return outs

    _chain_cache = {}

    def run_chain(dev_ins, k):
        """k sequential kernel executions inside one jit (outputs chained
        into the next call's donated buffers) — amortizes RPC overhead."""
        if k not in _chain_cache:
            def _chain(*args):
                ins = args[:n_params]
                outs = args[n_params:]
                for _ in range(k):
                    outs = _body(*ins, *outs)
                return outs
            _chain_cache[k] = jax.jit(
                shard_map(_chain, mesh=mesh, in_specs=in_specs,
                          out_specs=out_specs, check_rep=False),
                donate_argnums=donate, keep_unused=True)
        outs = _chain_cache[k](*dev_ins, *zmaker())
        jax.block_until_ready(outs)
        return outs

    return put_inputs, run, out_names, run_chain


def _prep_inputs(output_tensor, emb, W_ih0, W_hh0, b_ih0, b_hh0,
                 W_ih1, W_hh1, b_ih1, b_hh1, W_out, b_out, T1):
    """Build per-core in_maps (host-side sharding)."""
    tok = np.asarray(output_tensor)[:T1]
    x = emb[tok] * (tok != 0)[..., None].astype(np.float32)      # (T1,B,E)
    xT = np.ascontiguousarray(
        x.reshape(T1, B, 4, 128).transpose(0, 3, 2, 1)).reshape(T1, 128, 4 * B)

    def pack_w(Wsl, nch):
        # Wsl (512 gate rows, K) -> [p, c*GS+j] = Wsl[j, c*128+p]
        K = Wsl.shape[1]
        arr = Wsl.T.reshape(nch, 128, GS).transpose(1, 0, 2).reshape(128, nch * GS)
        assert K == nch * 128
        return np.ascontiguousarray(arr)

    ones_r = np.ones((1, B), np.float32)
    ones_b = np.ones((1, 128), ml_dtypes.bfloat16)
    ident = np.eye(64, dtype=np.float32)

    in_maps = []
    for k in range(NC):
        gr = np.concatenate([np.arange(q * H + k * HS, q * H + (k + 1) * HS)
                             for q in range(4)])
        vsl = slice(k * VS, (k + 1) * VS)
        woutT = (W_out[vsl].T.reshape(8, 128, VS).transpose(1, 0, 2)
                 .reshape(128, 8 * VS))
        in_maps.append({
            "xT": xT,
            "wih0": pack_w(W_ih0[gr], 4),
            "whh0": pack_w(W_hh0[gr], 8),
            "wih1": pack_w(W_ih1[gr], 8),
            "whh1": pack_w(W_hh1[gr], 8),
            "b0": (b_ih0 + b_hh0)[gr][None, :].astype(np.float32),
            "b1": (b_ih1 + b_hh1)[gr][None, :].astype(np.float32),
            "wout": np.ascontiguousarray(woutT).astype(ml_dtypes.bfloat16),
            "bout": b_out[vsl][None, :].astype(ml_dtypes.bfloat16),
            "ones_r": ones_r,
            "ones_b": ones_b,
            "ident": ident,
        })
    return in_maps


def _combine(s_all, i8_all, h1_core0, output_tensor, W_out, b_out, T, T1):
    """Host-side unshard: loss + greedy preds from per-core stats.

    s_all: (NC, NT, 128) sumexp per core; i8_all: (NC, NT, 128, 8) local idx;
    h1_core0: (T1, 128, 8, B) h1.T chunks from core 0.
    """
    Ntok = T1 * B
    NT = (Ntok + 127) // 128
    s_g = s_all.astype(np.float64).sum(axis=0).reshape(-1)
    lse = np.log(s_g[:Ntok]).reshape(T1, B)

    # h1 as the device used it (f32r-rounded): (T1,128,8,B)->(T1,B,H)
    h1 = h1_core0.transpose(0, 3, 2, 1).reshape(T1, B, H)
    h1 = np.ascontiguousarray(h1, np.float32)

    tgt = np.asarray(output_tensor)[1:T1 + 1]
    tv = (np.einsum("tbh,tbh->tb", h1.astype(np.float64),
                    W_out[tgt].astype(np.float64))
          + b_out[tgt].astype(np.float64))
    nll = lse - tv
    valid = (tgt != 1)
    denom = np.maximum(valid.sum(axis=1), 1.0)
    ce = (nll * valid).sum(axis=1) / denom
    loss = np.float32(ce.sum() / T)

    # candidate-refined argmax
    cand = np.zeros((NT * 128, NC * 8), np.int64)
    for k in range(NC):
        cand[:, k * 8:(k + 1) * 8] = (
            i8_all[k].reshape(-1, 8).astype(np.int64) + k * VS)
    cand = np.sort(cand[:Ntok], axis=1)
    h1f = h1.reshape(Ntok, H)
    preds = np.zeros((Ntok,), np.int64)
    CHUNK = 512
    for i in range(0, Ntok, CHUNK):
        c = cand[i:i + CHUNK]
        wl = W_out[c]                                   # (n, 64, H)
        cl = np.einsum("nkh,nh->nk", wl, h1f[i:i + CHUNK]) + b_out[c]
        preds[i:i + CHUNK] = c[np.arange(len(c)), np.argmax(cl, axis=1)]
    preds = preds.reshape(T1, B).astype(np.int32)
    result = np.concatenate(
        [np.full((1, B), 2, np.int32), preds], axis=0)
    return loss, result


def kernel(output_tensor, encoder_hidden_states=None, input_mask=None,
           hidden_state=None, cell_state=None, encoder_attention=None,
           max_length=None, emb=None, W_ih0=None, W_hh0=None, b_ih0=None,
           b_hh0=None, W_ih1=None, W_hh1=None, b_ih1=None, b_hh1=None,
           W_out=None, b_out=None):
    output_tensor = np.asarray(output_tensor)
    T = output_tensor.shape[0]
    T1 = T - 1
    emb = np.asarray(emb, np.float32)
    W_ih0 = np.asarray(W_ih0, np.float32); W_hh0 = np.asarray(W_hh0, np.float32)
    W_ih1 = np.asarray(W_ih1, np.float32); W_hh1 = np.asarray(W_hh1, np.float32)
    b_ih0 = np.asarray(b_ih0, np.float32); b_hh0 = np.asarray(b_hh0, np.float32)
    b_ih1 = np.asarray(b_ih1, np.float32); b_hh1 = np.asarray(b_hh1, np.float32)
    W_out = np.asarray(W_out, np.float32); b_out = np.asarray(b_out, np.float32)

    if T1 not in _BUILD_CACHE:
        _BUILD_CACHE[T1] = build(T1)
    nc = _BUILD_CACHE[T1]

    in_maps = _prep_inputs(output_tensor, emb, W_ih0, W_hh0, b_ih0, b_hh0,
                           W_ih1, W_hh1, b_ih1, b_hh1, W_out, b_out, T1)
    res = bass_utils.run_bass_kernel_spmd(nc, in_maps,
                                          core_ids=list(range(NC)))
    NT = (T1 * B + 127) // 128
    s_all = np.stack([r["s_out"] for r in res.results])
    i8_all = np.stack([r["i8_out"] for r in res.results])
    h1_core0 = res.results[0]["h1_out"]
    return _combine(s_all, i8_all, h1_core0, output_tensor, W_out, b_out,
                    T, T1)
